# revision 2
# baseline (speedup 1.0000x reference)
"""BiLIF (bidirectional leaky-integrate-and-fire) node on 8 Trainium2 NeuronCores.

Problem: inputs [T=16, B=64, N=65536] f32.
  s1 = LIF-scan(x,          tau=4/3, v_th=0.75)   (hard reset to 0)
  s2 = LIF-scan(flip(x, 0), tau=4/3, v_th=1.25)
  out = (s1 + s2) / 2

Strategy (v2)
  - Shard the batch dim across the 8 cores (pure data parallel). Per core:
    8*65536 positions = 128 partitions x 4096 cols, two 2048-col chunks.
  - Both direction scans run concurrently (fwd consumes x[t] at step t, bwd
    consumes x[15-t]), so out[t] completes at step t and every x tile is
    loaded exactly once.
  - The LIF step (reset -> charge) is one fused 2-src custom DVE op
    (measured 1 elem/lane/cycle @ ~1.4 GHz; this is the VectorE floor).
  - Output combine s1+s2 is split:
      * cols [0,256): VectorE fused OUT op -> fp8 {0,1,2} per element.
      * cols [256,2048): ScalarE Sign(h-th) -> bf16 sign tiles; TensorE
        packs 4 PARTITION rows per output byte with a [128x32] weight
        matrix W (W[k,p'] = 4^(k mod 4) on rows k//4 == p'), accumulating
        2 dirs x 4 consecutive time steps into one [128, 1792] PSUM tile
        at partition offsets 32*(t%4) (PE col-tiling); one ScalarE copy
        per 4 steps turns psum = 2P-170 into P = sum_r 4^r (s1+s2)[4p'+r]
        as exact uint8 in [0,170].
    This cuts the ScalarE psum-copy cost 4x and the packed output bytes 4x
    vs the per-element-fp8 version, rebalancing VectorE/ScalarE at ~102 us
    busy each; output DMA drops 8 MB -> ~3.7 MB per core.
  - Host unpacks base-4 digits and scales by 0.5.
"""

import numpy as np
import ml_dtypes  # noqa: F401  (fp8 dtype availability)

import concourse.bacc as bacc
import concourse.mybir as mybir
import concourse.tile as tile
import concourse.dve_ops as dve_ops
from concourse.dve_ops import DveOp
from concourse.dve_spec import (
    C0,
    C1,
    C2,
    Spec,
    Src0,
    Src1,
    Zero,
    _has_src1,
    lower,
    select,
)
from concourse.dve_uop import DveOpSpec
from concourse import bass_utils

T, B, N = 16, 64, 65536
NCORES = 8
BS = B // NCORES        # batch rows per core
POS = BS * N            # independent positions per core
P = 128
FREE = POS // P         # 4096 columns per partition
CHUNK = 2048
NCHUNK = FREE // CHUNK
DVE_COLS = 256          # columns combined by the VectorE fused OUT op
ACT_COLS = CHUNK - DVE_COLS  # columns via ScalarE+TensorE packed path
TGROUP = 4              # time steps packed per psum tile / output byte row
R = 0.75                # fl32(1 / fl32(4/3)) == 0.75 exactly
TH1, TH2 = 0.75, 1.25
F32 = mybir.dt.float32
BF16 = mybir.dt.bfloat16
FP8 = mybir.dt.float8e4
U8 = mybir.dt.uint8
AF = mybir.ActivationFunctionType


def _register(name: str, spec: Spec) -> DveOp:
    """Register a custom DVE op at runtime (uops sha computed here)."""
    if name in dve_ops._SUB_OPCODE_FOR_NAME:
        for op in dve_ops.OPS:
            if op.name == name:
                return op
    row = dve_ops._CUSTOM_DVE_ROW_BASE + len(dve_ops.OPS)
    assert row < 0x20, "custom DVE opcode rows exhausted"
    sha = {}
    for ver in ("v3", "v4"):
        s = DveOpSpec(name=name, opcode=row, uops=lower(spec, ver=ver),
                      rd1_en=_has_src1(spec))
        sha[ver] = s.sha(ver)
    op = DveOp(name, spec, subdim=False, uops_sha=sha)
    dve_ops.OPS.append(op)
    dve_ops._SUB_OPCODE_FOR_NAME[name] = row
    dve_ops.CUSTOM_DVE_SPECS[name] = spec
    return op


_vp_node = select(Src1 < C1, Src1, Zero)
BILIF_STEP = _register(
    "BILIF_STEP",
    Spec(
        body=(Src0 - _vp_node) * C0 + _vp_node,
        reference=lambda in0, in1, s0, s1, imm2: (
            (in0 - np.where(in1 < s1, in1, 0).astype(np.float32))
            * np.float32(s0)
            + np.where(in1 < s1, in1, 0).astype(np.float32)
        ),
    ),
)
BILIF_OUT = _register(
    "BILIF_OUT",
    Spec(
        body=((Src0 >= C0) + (Src1 >= C1)) * C2,
        reference=lambda in0, in1, s0, s1, imm2: (
            (in0 >= s0).astype(np.float32) + (in1 >= s1).astype(np.float32)
        ) * np.float32(imm2),
    ),
)


def make_pack_weights() -> np.ndarray:
    """[128, 32] bf16: W[k, p'] = 4^(k % 4) if k // 4 == p' else 0."""
    w = np.zeros((P, 32), np.float32)
    for k in range(P):
        w[k, k // 4] = float(4 ** (k % 4))
    return w.astype(ml_dtypes.bfloat16)


_NC_CACHE = {}


def _build_nc(repeat: int = 1):
    """Build + compile the SPMD per-core program. `repeat` replays the body
    (used only for steady-state timing experiments)."""
    if repeat in _NC_CACHE:
        return _NC_CACHE[repeat]
    nblk = (ACT_COLS + 511) // 512
    ngroup = T // TGROUP
    nc = bacc.Bacc("TRN2", target_bir_lowering=False, debug=False,
                   num_devices=NCORES)
    x_d = nc.dram_tensor("x", [T * P, FREE], F32, kind="ExternalInput").ap()
    w_d = nc.dram_tensor("w", [P, 32], BF16, kind="ExternalInput").ap()
    o_d = nc.dram_tensor("o", [T * P, NCHUNK * DVE_COLS], FP8,
                         kind="ExternalOutput").ap()
    p_d = nc.dram_tensor("p", [ngroup * P, NCHUNK * ACT_COLS], U8,
                         kind="ExternalOutput").ap()

    with tile.TileContext(nc) as tc:
        with tc.tile_pool(name="xp", bufs=16) as xp, \
             tc.tile_pool(name="h1p", bufs=3) as h1p, \
             tc.tile_pool(name="h2p", bufs=3) as h2p, \
             tc.tile_pool(name="a1p", bufs=2) as a1p, \
             tc.tile_pool(name="a2p", bufs=2) as a2p, \
             tc.tile_pool(name="outp", bufs=4) as outp, \
             tc.tile_pool(name="pkp", bufs=2) as pkp, \
             tc.tile_pool(name="psp", bufs=2, space="PSUM") as psp, \
             tc.tile_pool(name="zp", bufs=1) as zp:
            wt = zp.tile([P, 32], BF16, tag="w", name="w")
            nc.sync.dma_start(out=wt[:], in_=w_d[:, :])
            b1 = zp.tile([P, 1], F32, tag="b1", name="b1")
            nc.vector.memset(b1[:], -TH1)
            b2 = zp.tile([P, 1], F32, tag="b2", name="b2")
            nc.vector.memset(b2[:], -TH2)
            for rep in range(repeat):
                for k in range(NCHUNK):
                    c0 = k * CHUNK
                    # Load each x[t] tile once, in first-use order
                    # (fwd uses t at step t, bwd uses t at step 15-t).
                    xt = {}
                    for t in [v for s in range(T // 2) for v in (s, T - 1 - s)]:
                        xt[t] = xp.tile([P, CHUNK], F32, tag="x",
                                        name=f"x{rep}_{k}_{t}")
                        nc.sync.dma_start(
                            out=xt[t][:],
                            in_=x_d[t * P:(t + 1) * P, c0:c0 + CHUNK])
                    h1_prev, h2_prev = None, None
                    ps = None
                    for t in range(T):
                        h1 = h1p.tile([P, CHUNK], F32, tag="h1", name="h1")
                        h2 = h2p.tile([P, CHUNK], F32, tag="h2", name="h2")
                        if t == 0:
                            # v = 0: h = 0.75*x exactly; single-src fp32
                            # tensor_scalar streams at 2x (2-port mode)
                            nc.vector.tensor_scalar(
                                out=h1[:], in0=xt[0][:], scalar1=R,
                                scalar2=None, op0=mybir.AluOpType.mult)
                            nc.vector.tensor_scalar(
                                out=h2[:], in0=xt[T - 1][:], scalar1=R,
                                scalar2=None, op0=mybir.AluOpType.mult)
                        else:
                            nc.vector._custom_dve(BILIF_STEP, out=h1[:],
                                                  in0=xt[t][:],
                                                  in1=h1_prev[:],
                                                  s0=R, s1=TH1)
                            nc.vector._custom_dve(BILIF_STEP, out=h2[:],
                                                  in0=xt[T - 1 - t][:],
                                                  in1=h2_prev[:],
                                                  s0=R, s1=TH2)
                        # VectorE-direct combine for the first DVE_COLS
                        o = outp.tile([P, DVE_COLS], FP8, tag="o", name="o")
                        nc.vector._custom_dve(
                            BILIF_OUT, out=o[:],
                            in0=h1[:, :DVE_COLS], in1=h2[:, :DVE_COLS],
                            s0=TH1, s1=TH2, imm2=1.0)
                        nc.sync.dma_start(
                            out=o_d[t * P:(t + 1) * P,
                                    k * DVE_COLS:(k + 1) * DVE_COLS],
                            in_=o[:])
                        # ScalarE+TensorE packed path for the rest
                        m = t % TGROUP
                        if m == 0:
                            ps = psp.tile([P, ACT_COLS], F32, tag="ps",
                                          name="ps")
                        a1 = a1p.tile([P, ACT_COLS], BF16, tag="a1",
                                      name="a1")
                        nc.scalar.activation(out=a1[:],
                                             in_=h1[:, DVE_COLS:],
                                             func=AF.Sign, bias=b1[:],
                                             scale=1.0)
                        a2 = a2p.tile([P, ACT_COLS], BF16, tag="a2",
                                      name="a2")
                        nc.scalar.activation(out=a2[:],
                                             in_=h2[:, DVE_COLS:],
                                             func=AF.Sign, bias=b2[:],
                                             scale=1.0)
                        for j in range(nblk):
                            sl = slice(j * 512, min((j + 1) * 512, ACT_COLS))
                            nc.tensor.matmul(ps[32 * m:32 * m + 32, sl],
                                             wt[:], a1[:, sl], start=True,
                                             stop=False,
                                             tile_position=(0, 32 * m))
                            nc.tensor.matmul(ps[32 * m:32 * m + 32, sl],
                                             wt[:], a2[:, sl], start=False,
                                             stop=True,
                                             tile_position=(0, 32 * m))
                        if m == TGROUP - 1:
                            # psum rows 32m' hold 2*P' - 170 for step
                            # t = g*TGROUP + m'; 0.5*psum + 85 = P' exactly.
                            g = t // TGROUP
                            pk = pkp.tile([P, ACT_COLS], U8, tag="pk",
                                          name="pk")
                            nc.scalar.activation(out=pk[:], in_=ps[:],
                                                 func=AF.Copy,
                                                 bias=85.0, scale=0.5)
                            nc.sync.dma_start(
                                out=p_d[g * P:(g + 1) * P,
                                        k * ACT_COLS:(k + 1) * ACT_COLS],
                                in_=pk[:])
                        h1_prev, h2_prev = h1, h2

    nc.compile()
    _NC_CACHE[repeat] = nc
    return nc


def _run(inputs: np.ndarray, repeat: int = 1, **kwargs):
    nc = _build_nc(repeat)
    w = make_pack_weights()
    in_maps = []
    for c in range(NCORES):
        shard = np.ascontiguousarray(
            inputs[:, c * BS:(c + 1) * BS, :]).reshape(T * P, FREE)
        in_maps.append({"x": shard, "w": w})
    return bass_utils.run_bass_kernel_spmd(
        nc, in_maps, core_ids=list(range(NCORES)), **kwargs)


def _decode_core(o_fp8: np.ndarray, p_u8: np.ndarray) -> np.ndarray:
    """Reassemble one core's [T*P, FREE] f32 output (= (s1+s2)/2)."""
    out = np.empty((T, P, FREE), np.float32)
    q_dve = o_fp8.astype(np.float32).reshape(T, P, NCHUNK, DVE_COLS)
    pk = p_u8.reshape(T // TGROUP, TGROUP, 32, NCHUNK, ACT_COLS)
    # pk[g, m, p', k, c] packs q for partition rows 4p'+r, step 4g+m
    digits = np.empty((T // TGROUP, TGROUP, 32, 4, NCHUNK, ACT_COLS),
                      np.uint8)
    v = pk
    for r in range(4):
        digits[:, :, :, r] = v % 4
        v = v // 4
    q_act = digits.reshape(T // TGROUP, TGROUP, P, NCHUNK, ACT_COLS)
    q_act = q_act.reshape(T, P, NCHUNK, ACT_COLS)
    for k in range(NCHUNK):
        out[:, :, k * CHUNK:k * CHUNK + DVE_COLS] = q_dve[:, :, k]
        out[:, :, k * CHUNK + DVE_COLS:(k + 1) * CHUNK] = q_act[:, :, k]
    return out.reshape(T * P, FREE) * np.float32(0.5)


def kernel(inputs: np.ndarray, **kwargs) -> np.ndarray:
    inputs = np.asarray(inputs)
    assert inputs.shape == (T, B, N) and inputs.dtype == np.float32
    res = None
    err = None
    for _attempt in range(3):  # retry transient device faults
        try:
            res = _run(inputs, **kwargs)
            break
        except Exception as e:  # noqa: BLE001
            err = e
    if res is None:
        raise err
    out = np.empty((T, B, N), np.float32)
    for c in range(NCORES):
        dec = _decode_core(np.asarray(res.results[c]["o"]),
                           np.asarray(res.results[c]["p"]))
        out[:, c * BS:(c + 1) * BS, :] = dec.reshape(T, BS, N)
    return out


# revision 4
# speedup vs baseline: 1.4051x; 1.4051x over previous
"""BiLIF (bidirectional leaky-integrate-and-fire) node on 8 Trainium2 NeuronCores.

Problem: inputs [T=16, B=64, N=65536] f32.
  s1 = LIF-scan(x,          tau=4/3, v_th=0.75)   (hard reset to 0)
  s2 = LIF-scan(flip(x, 0), tau=4/3, v_th=1.25)
  out = (s1 + s2) / 2

Strategy (v2)
  - Shard the batch dim across the 8 cores (pure data parallel). Per core:
    8*65536 positions = 128 partitions x 4096 cols, two 2048-col chunks.
  - Both direction scans run concurrently (fwd consumes x[t] at step t, bwd
    consumes x[15-t]), so out[t] completes at step t and every x tile is
    loaded exactly once.
  - The LIF step (reset -> charge) is one fused 2-src custom DVE op
    (measured 1 elem/lane/cycle @ ~1.4 GHz; this is the VectorE floor).
  - Output combine s1+s2 is split:
      * cols [0,256): VectorE fused OUT op -> fp8 {0,1,2} per element.
      * cols [256,2048): ScalarE Sign(h-th) -> bf16 sign tiles; TensorE
        packs 4 PARTITION rows per output byte with a [128x32] weight
        matrix W (W[k,p'] = 4^(k mod 4) on rows k//4 == p'), accumulating
        2 dirs x 4 consecutive time steps into one [128, 1792] PSUM tile
        at partition offsets 32*(t%4) (PE col-tiling); one ScalarE copy
        per 4 steps turns psum = 2P-170 into P = sum_r 4^r (s1+s2)[4p'+r]
        as exact uint8 in [0,170].
    This cuts the ScalarE psum-copy cost 4x and the packed output bytes 4x
    vs the per-element-fp8 version, rebalancing VectorE/ScalarE at ~102 us
    busy each; output DMA drops 8 MB -> ~3.7 MB per core.
  - Host unpacks base-4 digits and scales by 0.5.
"""

import numpy as np
import ml_dtypes  # noqa: F401  (fp8 dtype availability)

import concourse.bacc as bacc
import concourse.mybir as mybir
import concourse.tile as tile
import concourse.dve_ops as dve_ops
from concourse.dve_ops import DveOp
from concourse.dve_spec import (
    C0,
    C1,
    C2,
    Spec,
    Src0,
    Src1,
    Zero,
    _has_src1,
    lower,
    select,
)
from concourse.dve_uop import DveOpSpec
from concourse import bass_utils

T, B, N = 16, 64, 65536
NCORES = 8
BS = B // NCORES        # batch rows per core
POS = BS * N            # independent positions per core
P = 128
FREE = POS // P         # 4096 columns per partition
CHUNK = 2048
NCHUNK = FREE // CHUNK
DVE_COLS = 256          # columns combined by the VectorE fused OUT op
ACT_COLS = CHUNK - DVE_COLS  # columns via ScalarE+TensorE packed path
TGROUP = 4              # time steps packed per psum tile / output byte row
R = 0.75                # fl32(1 / fl32(4/3)) == 0.75 exactly
TH1, TH2 = 0.75, 1.25
F32 = mybir.dt.float32
BF16 = mybir.dt.bfloat16
FP8 = mybir.dt.float8e4
U8 = mybir.dt.uint8
AF = mybir.ActivationFunctionType


def _register(name: str, spec: Spec) -> DveOp:
    """Register a custom DVE op at runtime (uops sha computed here)."""
    if name in dve_ops._SUB_OPCODE_FOR_NAME:
        for op in dve_ops.OPS:
            if op.name == name:
                return op
    row = dve_ops._CUSTOM_DVE_ROW_BASE + len(dve_ops.OPS)
    assert row < 0x20, "custom DVE opcode rows exhausted"
    sha = {}
    for ver in ("v3", "v4"):
        s = DveOpSpec(name=name, opcode=row, uops=lower(spec, ver=ver),
                      rd1_en=_has_src1(spec))
        sha[ver] = s.sha(ver)
    op = DveOp(name, spec, subdim=False, uops_sha=sha)
    dve_ops.OPS.append(op)
    dve_ops._SUB_OPCODE_FOR_NAME[name] = row
    dve_ops.CUSTOM_DVE_SPECS[name] = spec
    return op


_vp_node = select(Src1 < C1, Src1, Zero)
BILIF_STEP = _register(
    "BILIF_STEP",
    Spec(
        body=(Src0 - _vp_node) * C0 + _vp_node,
        reference=lambda in0, in1, s0, s1, imm2: (
            (in0 - np.where(in1 < s1, in1, 0).astype(np.float32))
            * np.float32(s0)
            + np.where(in1 < s1, in1, 0).astype(np.float32)
        ),
    ),
)
BILIF_OUT = _register(
    "BILIF_OUT",
    Spec(
        body=((Src0 >= C0) + (Src1 >= C1)) * C2,
        reference=lambda in0, in1, s0, s1, imm2: (
            (in0 >= s0).astype(np.float32) + (in1 >= s1).astype(np.float32)
        ) * np.float32(imm2),
    ),
)


def make_pack_weights() -> np.ndarray:
    """[128, 32] bf16: W[k, p'] = 4^(k % 4) if k // 4 == p' else 0."""
    w = np.zeros((P, 32), np.float32)
    for k in range(P):
        w[k, k // 4] = float(4 ** (k % 4))
    return w.astype(ml_dtypes.bfloat16)


_NC_CACHE = {}


def _build_nc(repeat: int = 1):
    """Build + compile the SPMD per-core program. `repeat` replays the body
    (used only for steady-state timing experiments)."""
    if repeat in _NC_CACHE:
        return _NC_CACHE[repeat]
    nblk = (ACT_COLS + 511) // 512
    ngroup = T // TGROUP
    nc = bacc.Bacc("TRN2", target_bir_lowering=False, debug=False,
                   num_devices=NCORES)
    x_d = nc.dram_tensor("x", [T * P, FREE], F32, kind="ExternalInput").ap()
    w_d = nc.dram_tensor("w", [P, 32], BF16, kind="ExternalInput").ap()
    o_d = nc.dram_tensor("o", [T * P, NCHUNK * DVE_COLS], FP8,
                         kind="ExternalOutput").ap()
    p_d = nc.dram_tensor("p", [ngroup * P, NCHUNK * ACT_COLS], U8,
                         kind="ExternalOutput").ap()

    with tile.TileContext(nc) as tc:
        with tc.tile_pool(name="xp", bufs=16) as xp, \
             tc.tile_pool(name="h1p", bufs=3) as h1p, \
             tc.tile_pool(name="h2p", bufs=3) as h2p, \
             tc.tile_pool(name="a1p", bufs=2) as a1p, \
             tc.tile_pool(name="a2p", bufs=2) as a2p, \
             tc.tile_pool(name="outp", bufs=4) as outp, \
             tc.tile_pool(name="pkp", bufs=2) as pkp, \
             tc.tile_pool(name="psp", bufs=2, space="PSUM") as psp, \
             tc.tile_pool(name="zp", bufs=1) as zp:
            wt = zp.tile([P, 32], BF16, tag="w", name="w")
            nc.sync.dma_start(out=wt[:], in_=w_d[:, :])
            b1 = zp.tile([P, 1], F32, tag="b1", name="b1")
            nc.vector.memset(b1[:], -TH1)
            b2 = zp.tile([P, 1], F32, tag="b2", name="b2")
            nc.vector.memset(b2[:], -TH2)
            for rep in range(repeat):
                for k in range(NCHUNK):
                    c0 = k * CHUNK
                    # Load each x[t] tile once, in first-use order
                    # (fwd uses t at step t, bwd uses t at step 15-t).
                    xt = {}
                    for t in [v for s in range(T // 2) for v in (s, T - 1 - s)]:
                        xt[t] = xp.tile([P, CHUNK], F32, tag="x",
                                        name=f"x{rep}_{k}_{t}")
                        nc.sync.dma_start(
                            out=xt[t][:],
                            in_=x_d[t * P:(t + 1) * P, c0:c0 + CHUNK])
                    h1_prev, h2_prev = None, None
                    ps = None
                    for t in range(T):
                        h1 = h1p.tile([P, CHUNK], F32, tag="h1", name="h1")
                        h2 = h2p.tile([P, CHUNK], F32, tag="h2", name="h2")
                        if t == 0:
                            # v = 0: h = 0.75*x exactly; single-src fp32
                            # tensor_scalar streams at 2x (2-port mode)
                            nc.vector.tensor_scalar(
                                out=h1[:], in0=xt[0][:], scalar1=R,
                                scalar2=None, op0=mybir.AluOpType.mult)
                            nc.vector.tensor_scalar(
                                out=h2[:], in0=xt[T - 1][:], scalar1=R,
                                scalar2=None, op0=mybir.AluOpType.mult)
                        else:
                            nc.vector._custom_dve(BILIF_STEP, out=h1[:],
                                                  in0=xt[t][:],
                                                  in1=h1_prev[:],
                                                  s0=R, s1=TH1)
                            nc.vector._custom_dve(BILIF_STEP, out=h2[:],
                                                  in0=xt[T - 1 - t][:],
                                                  in1=h2_prev[:],
                                                  s0=R, s1=TH2)
                        # VectorE-direct combine for the first DVE_COLS
                        o = outp.tile([P, DVE_COLS], FP8, tag="o", name="o")
                        nc.vector._custom_dve(
                            BILIF_OUT, out=o[:],
                            in0=h1[:, :DVE_COLS], in1=h2[:, :DVE_COLS],
                            s0=TH1, s1=TH2, imm2=1.0)
                        nc.sync.dma_start(
                            out=o_d[t * P:(t + 1) * P,
                                    k * DVE_COLS:(k + 1) * DVE_COLS],
                            in_=o[:])
                        # ScalarE+TensorE packed path for the rest
                        m = t % TGROUP
                        if m == 0:
                            ps = psp.tile([P, ACT_COLS], F32, tag="ps",
                                          name="ps")
                        a1 = a1p.tile([P, ACT_COLS], BF16, tag="a1",
                                      name="a1")
                        nc.scalar.activation(out=a1[:],
                                             in_=h1[:, DVE_COLS:],
                                             func=AF.Sign, bias=b1[:],
                                             scale=1.0)
                        a2 = a2p.tile([P, ACT_COLS], BF16, tag="a2",
                                      name="a2")
                        nc.scalar.activation(out=a2[:],
                                             in_=h2[:, DVE_COLS:],
                                             func=AF.Sign, bias=b2[:],
                                             scale=1.0)
                        for j in range(nblk):
                            sl = slice(j * 512, min((j + 1) * 512, ACT_COLS))
                            nc.tensor.matmul(ps[32 * m:32 * m + 32, sl],
                                             wt[:], a1[:, sl], start=True,
                                             stop=False,
                                             tile_position=(0, 32 * m))
                            nc.tensor.matmul(ps[32 * m:32 * m + 32, sl],
                                             wt[:], a2[:, sl], start=False,
                                             stop=True,
                                             tile_position=(0, 32 * m))
                        if m == TGROUP - 1:
                            # psum rows 32m' hold 2*P' - 170 for step
                            # t = g*TGROUP + m'; 0.5*psum + 85 = P' exactly.
                            g = t // TGROUP
                            pk = pkp.tile([P, ACT_COLS], U8, tag="pk",
                                          name="pk")
                            nc.scalar.activation(out=pk[:], in_=ps[:],
                                                 func=AF.Copy,
                                                 bias=85.0, scale=0.5)
                            nc.sync.dma_start(
                                out=p_d[g * P:(g + 1) * P,
                                        k * ACT_COLS:(k + 1) * ACT_COLS],
                                in_=pk[:])
                        h1_prev, h2_prev = h1, h2

    nc.compile()
    _NC_CACHE[repeat] = nc
    return nc


def _run(inputs: np.ndarray, repeat: int = 1, **kwargs):
    nc = _build_nc(repeat)
    w = make_pack_weights()
    in_maps = []
    for c in range(NCORES):
        shard = np.ascontiguousarray(
            inputs[:, c * BS:(c + 1) * BS, :]).reshape(T * P, FREE)
        in_maps.append({"x": shard, "w": w})
    return bass_utils.run_bass_kernel_spmd(
        nc, in_maps, core_ids=list(range(NCORES)), **kwargs)


def _decode_core(o_fp8: np.ndarray, p_u8: np.ndarray) -> np.ndarray:
    """Reassemble one core's [T*P, FREE] f32 output (= (s1+s2)/2)."""
    out = np.empty((T, P, FREE), np.float32)
    q_dve = o_fp8.astype(np.float32).reshape(T, P, NCHUNK, DVE_COLS)
    pk = p_u8.reshape(T // TGROUP, TGROUP, 32, NCHUNK, ACT_COLS)
    # pk[g, m, p', k, c] packs q for partition rows 4p'+r, step 4g+m
    digits = np.empty((T // TGROUP, TGROUP, 32, 4, NCHUNK, ACT_COLS),
                      np.uint8)
    v = pk
    for r in range(4):
        digits[:, :, :, r] = v % 4
        v = v // 4
    q_act = digits.reshape(T // TGROUP, TGROUP, P, NCHUNK, ACT_COLS)
    q_act = q_act.reshape(T, P, NCHUNK, ACT_COLS)
    for k in range(NCHUNK):
        out[:, :, k * CHUNK:k * CHUNK + DVE_COLS] = q_dve[:, :, k]
        out[:, :, k * CHUNK + DVE_COLS:(k + 1) * CHUNK] = q_act[:, :, k]
    return out.reshape(T * P, FREE) * np.float32(0.5)


def kernel(inputs: np.ndarray, **kwargs) -> np.ndarray:
    inputs = np.asarray(inputs)
    assert inputs.shape == (T, B, N) and inputs.dtype == np.float32
    res = None
    err = None
    for _attempt in range(3):  # retry transient device faults
        try:
            res = _run(inputs, **kwargs)
            break
        except Exception as e:  # noqa: BLE001
            err = e
    if res is None:
        raise err
    out = np.empty((T, B, N), np.float32)
    for c in range(NCORES):
        dec = _decode_core(np.asarray(res.results[c]["o"]),
                           np.asarray(res.results[c]["p"]))
        out[:, c * BS:(c + 1) * BS, :] = dec.reshape(T, BS, N)
    return out
